# revision 45
# baseline (speedup 1.0000x reference)
"""Trainium2 Bass kernel for nn_AET_34737695490187 (histogram_binning).

Math (per sample):
  d = bbox // 72; label = y0*14+x0 where (x0==x1 & y0==y1 & mask) else invalid
  segment-sum text_embeds into 196 patch buckets -> sums, counts
  tpe = sums / max(counts, 1)
  logits1 = ipe @ tpe^T (per sample); logits2 = logits1^T
  loss = mean of CE(logits, diag) both ways / 2

Key observation: validity requires x0==x1 & y0==y1 & mask, true for ~1/392
of tokens (~10-20 per 8-sample shard). Streaming all of text_embeds (and
the dense 512x196 scatter matmul) is ~99% wasted HBM traffic / PE columns.
This kernel instead:

  1. computes validity + bucket labels on-device from bbox (tiny),
  2. compacts valid tokens into <=128 global slots: per-partition
     local_scatter into slot columns, then a ones-matmul collapses
     partitions (each slot column has exactly one contributor),
  3. indirect-DMA-gathers only those embedding rows from DRAM,
  4. forms per-slot bucket sums with a slot-equality matrix EQ
     (duplicate buckets handled exactly: every slot of a bucket carries
     the full bucket sum and weight w=1/count, so per-bucket sums become
     w-weighted slot sums),
  5. computes logits [slot, 196] per sample with the bucket sums
     stationary (one 128-wide LDWEIGHTS per c-chunk, shared by all
     8 samples via separate psum banks),
  6. CE via zero structure: empty buckets contribute exp(0)=1
     (orientation 1) resp. ln(196) (orientation 2), so only slot
     rows/columns are exponentiated. No per-row max: logits are clamped
     at 83 (validated on data: ~1.5e-3 rel err vs 2e-2 tolerance).
  7. all Exps run before all Lns (one activation-table reload); the two
     final Lns are packed into ONE activation over [r2safe | r1mat/ones]
     with the accumulator contamination subtracted by a single STT.

Scheduling (measured on HW, 47.9us -> ~38us):
  - every small constant (triangular/identity masks, iota ramps,
    slot-select patterns) is generated on-device with gpsimd
    iota/affine_select/memset -- DMA'd constants queue behind the 2.4MB
    ipe stream and land 10+us late (per-ring FIFO, cold rings ~3.5us).
  - bbox/mask go FIRST on the warm sync ring, preceded by a tiny warm-up
    transfer (one DMA engine wakes ~2.4us late; the semaphore waits on
    the slowest of all 16 engines).
  - the valid-prefix-scan is 5 shifted adds on the vector engine (padded
    zero region) instead of PE transpose -> tri matmul -> transpose.
  - the row id for the gather is scattered as one fp16 value (4m+j-512)
    so rows = 512*k + mj needs a single int32-writing STT; empty slots
    dedupe to row 0 (masked downstream by w2sel/indmat).
  - the indirect gather is split in two 64-row halves so descriptor
    generation overlaps the first half's transfer.
  - logits run k-major with each sample's post chain (xs -> exp -> pick)
    issued immediately, pipelining V/S/G engines against the PE-bound
    logits+c1 matmul stream; the c1 matmul moves only P+2 columns (the
    diag-picked logit is pre-reduced into one column).
  - no gpsimd DVE op may interleave with local_scatter/indirect-DMA:
    each gpsimd library swap stalls its queue ~3us.

Sharding: pure data parallel, 8 samples per core on 8 NeuronCores.
"""

import os

import numpy as np

KDBG = bool(os.environ.get("KDBG"))

B, L, C, P = 64, 512, 768, 196
NCORES = 8
SPC = B // NCORES          # samples per core
TPD = 4                    # tokens per partition-col block (512 = 128*4)
NTOK = SPC * TPD           # 32 token cols per partition
NSLOT = 128                # global compacted-slot capacity per core
CLAMP = 83.0
SHIFT = 45.0                # exp/ln range shift: ACT Ln breaks above ~2^64
EM45 = float(np.exp(-45.0))
LN196 = float(np.log(196.0))

_NC_CACHE = None


def _build_nc():
    global _NC_CACHE
    if _NC_CACHE is not None:
        return _NC_CACHE

    import concourse.bass as bass
    import concourse.mybir as mybir
    from concourse import bacc, tile

    f32 = mybir.dt.float32
    bf16 = mybir.dt.bfloat16
    i32 = mybir.dt.int32
    i16 = mybir.dt.int16
    AF = mybir.ActivationFunctionType
    OP = mybir.AluOpType
    AX = mybir.AxisListType
    PSUM = bass.MemorySpace.PSUM

    nc = bacc.Bacc(target_bir_lowering=False, debug=False)

    ter_in = nc.dram_tensor("ter", [SPC * L, C], bf16, kind="ExternalInput")
    ipe_in = nc.dram_tensor("ipes", [128, SPC * 6 * P], bf16, kind="ExternalInput")
    bbox_in = nc.dram_tensor("bboxq", [128, SPC * 16], i32, kind="ExternalInput")
    mask_in = nc.dram_tensor("maskq", [128, NTOK], f32, kind="ExternalInput")
    out_ext = nc.dram_tensor("out", [1, 1], f32, kind="ExternalOutput")
    if KDBG:
        dbg_cmp = nc.dram_tensor("dbg_cmp", [128, 4], f32, kind="ExternalOutput")
        dbg_w = nc.dram_tensor("dbg_w", [128, 8], f32, kind="ExternalOutput")
        dbg_eq = nc.dram_tensor("dbg_eq", [128, 128], bf16, kind="ExternalOutput")
        dbg_xs = nc.dram_tensor("dbg_xs", [128, P], bf16, kind="ExternalOutput")
        dbg_eg = nc.dram_tensor("dbg_eg", [128, C], bf16, kind="ExternalOutput")
        dbg_c1 = nc.dram_tensor("dbg_c1", [SPC, P + 2], f32,
                                kind="ExternalOutput")
        dbg_ex = nc.dram_tensor("dbg_ex", [128, SPC], f32, kind="ExternalOutput")
        dbg_fin = nc.dram_tensor("dbg_fin", [128, 8], f32, kind="ExternalOutput")
    with tile.TileContext(nc) as tc:
        with (
            tc.tile_pool(name="const", bufs=1) as cpool,
            tc.tile_pool(name="ipe", bufs=SPC) as ipool,
            tc.tile_pool(name="seg", bufs=1) as segpool,
            tc.tile_pool(name="sml", bufs=1) as smlpool,
            tc.tile_pool(name="xs", bufs=3) as xspool,
            tc.tile_pool(name="ex", bufs=3) as expool,
            tc.tile_pool(name="scr", bufs=4) as scrpool,
            tc.tile_pool(name="acc", bufs=1) as accpool,
            tc.tile_pool(name="ps_a", bufs=1, space=PSUM) as ps_a,
            tc.tile_pool(name="ps_lg", bufs=2, space=PSUM) as ps_lg,
            tc.tile_pool(name="ps_c1", bufs=2, space=PSUM) as ps_c1,
        ):
            # ---- DMA in: bbox/mask FIRST on the sync ring, ahead of the ipe
            # stream. The sync ring is warm (template uses it) and packets
            # drain FIFO per ring, so these land in ~2us; on any other ring
            # their packets interleave 1:1 with the 1us-sized ipe packets
            # and take 10us+ (measured). A tiny warm-up transfer goes first:
            # some DMA engines take ~2.4us to wake from their doorbell, and
            # bbox's semaphore waits on the slowest of all 16. ----
            dwarm = cpool.tile([128, 1], i32, tag="dwarm")
            nc.sync.dma_start(dwarm[:], bbox_in[:, 0:1])
            bboxa = cpool.tile([128, SPC * 16], i32, tag="bboxa")
            nc.sync.dma_start(bboxa[:], bbox_in[:])
            maska = cpool.tile([128, NTOK], f32, tag="maska")
            nc.sync.dma_start(maska[:], mask_in[:])

            # big ipe stream: two halves on the sync ring, right behind
            ipeh = []
            for g in range(2):
                t = ipool.tile([128, 4 * 6 * P], bf16, name=f"ipeh{g}", tag="ipet")
                nc.sync.dma_start(t[:], ipe_in[:, g * 4 * 6 * P:(g + 1) * 4 * 6 * P])
                ipeh.append(t)
            ipet = [ipeh[k // 4][:, (k % 4) * 6 * P:(k % 4 + 1) * 6 * P]
                    for k in range(SPC)]

            # ---- on-device constants (gpsimd; no DMA) ----
            onesc = cpool.tile([128, 2], f32, tag="onesc")
            nc.gpsimd.memset(onesc[:], 1.0)
            onesf = onesc[:, 0:1]
            onesb = cpool.tile([128, 1], bf16, tag="onesb")
            nc.gpsimd.memset(onesb[:], 1.0)
            negsh = cpool.tile([128, 1], f32, tag="negsh")
            nc.gpsimd.memset(negsh[:], -SHIFT)

            # PE warmup source, needed first
            wsrc = cpool.tile([128, 128], bf16, tag="wsrc")
            nc.gpsimd.memset(wsrc[:], 0.0)

            # packed-Ln staging area: col 0 <- r2safe, cols 1:P+1 rows 0:8
            # <- r1mat; everything else stays 1.0 (ln(1)=0, accum-neutral)
            lnpack = cpool.tile([128, P + 1], f32, tag="lnpack")
            nc.gpsimd.memset(lnpack[:], 1.0)

            # strict-lower-as-columns mask: ltri[r, c] = 1 iff r < c
            ltric = cpool.tile([128, 128], bf16, tag="ltri")
            nc.gpsimd.memset(ltric[:], 1.0)
            nc.gpsimd.affine_select(
                out=ltric[:], in_=ltric[:], compare_op=OP.is_gt, fill=0.0,
                base=0, pattern=[[1, 128]], channel_multiplier=-1)

            # svals layout: [mj (fp16 bits) | k+1 | bucket | valid], 16-bit.
            # mj = 4m + j - 512 in fp16 (exact: |mj| <= 512 < 2048); the
            # token's ter row is then k_c*512 + mj with ter holding the 8
            # samples back-to-back (no zero block). Empty slots zero-fill
            # to mj=0, k_c=0 -> row 0: a harmless duplicate gather whose
            # slots are masked out downstream (w2sel/indmat are 0 there).
            f16 = mybir.dt.float16
            svals = smlpool.tile([128, 4 * NTOK], bf16, tag="svals")
            nc.gpsimd.iota(svals[:, 0:NTOK].bitcast(f16), [[0, SPC], [1, TPD]],
                           base=-512, channel_multiplier=4,
                           allow_small_or_imprecise_dtypes=True)
            nc.gpsimd.iota(svals[:, NTOK:2 * NTOK], [[1, SPC], [0, TPD]],
                           base=1, channel_multiplier=0,
                           allow_small_or_imprecise_dtypes=True)

            # fp32 identity (PE transpose helper for sege)
            idtc = cpool.tile([128, 128], f32, tag="idt")
            nc.gpsimd.memset(idtc[:], 1.0)
            nc.gpsimd.affine_select(
                out=idtc[:], in_=idtc[:], compare_op=OP.is_equal, fill=0.0,
                base=0, pattern=[[-1, 128]], channel_multiplier=1)

            # iotap[p, i] = i  (fp32 ramp 0..P-1)
            iotap = cpool.tile([128, P], f32, tag="iotap")
            nc.gpsimd.iota(iotap[:], [[1, P]], channel_multiplier=0,
                           allow_small_or_imprecise_dtypes=True)

            # colc[p, 8k+j] = 9(k+1) if j == k else -1  (w2sel selector)
            colcc = cpool.tile([128, 8 * SPC], f32, tag="colc")
            nc.gpsimd.iota(colcc[:], [[9, SPC], [0, 8]], base=9,
                           channel_multiplier=0,
                           allow_small_or_imprecise_dtypes=True)
            nc.gpsimd.affine_select(
                out=colcc[:], in_=colcc[:], compare_op=OP.is_equal, fill=-1.0,
                base=0, pattern=[[-1, SPC], [1, 8]], channel_multiplier=0)

            # warm the Exp activation table off the critical path. (Warming
            # Ln too does NOT help: the compiler reloads the table on every
            # Exp<->Ln switch, so an early Ln just inserts an extra 1.28us
            # Exp reload before the post-phase Exps -- measured.)
            wdum = smlpool.tile([1, 1], f32, tag="wdum")
            nc.scalar.activation(wdum[:], onesc[0:1, 0:1], AF.Exp)

            # PE HAM warmup: keep TensorE awake until the first real matmul
            wps = ps_a.tile([128, 128], f32, tag="a")
            NWARM = 14
            for wi in range(NWARM):
                nc.tensor.matmul(wps[:], wsrc[:], wsrc[:],
                                 start=(wi == 0), stop=(wi == NWARM - 1))

            # ---- seg chain: validity + bucket per token ----
            # d = bbox // 72 via (bbox * 1821) >> 17 (dual-op arith+bitwise
            # is rejected by the BIR verifier, so two instructions)
            mtile = segpool.tile([128, SPC * 16], i32, tag="mtile")
            nc.vector.tensor_scalar(mtile[:], bboxa[:], 1821, None, OP.mult)
            dtile = segpool.tile([128, SPC * 16], i32, tag="dtile")
            nc.vector.tensor_scalar(dtile[:], mtile[:], 17, None,
                                    OP.arith_shift_right)
            d3 = dtile[:].rearrange("p (t c) -> p t c", c=4)

            eqs = segpool.tile([128, 2 * NTOK], f32, tag="eqs")
            eqs3 = eqs[:].rearrange("p (t c) -> p t c", c=2)
            nc.vector.tensor_tensor(eqs3, d3[:, :, 0:2], d3[:, :, 2:4], OP.is_equal)

            v1 = segpool.tile([128, NTOK], f32, tag="v1")
            v1v = v1[:].rearrange("p (t o) -> p t o", o=1)
            nc.vector.tensor_tensor(v1v, eqs3[:, :, 0:1], eqs3[:, :, 1:2], OP.mult)
            valid = svals[:, 3 * NTOK:4 * NTOK]
            nc.vector.tensor_tensor(valid, v1[:], maska[:], OP.mult)

            buckv = (svals[:, 2 * NTOK:3 * NTOK]
                     .rearrange("p (t o) -> p t o", o=1))
            nc.vector.scalar_tensor_tensor(
                buckv, d3[:, :, 1:2], 14.0, d3[:, :, 0:1], OP.mult, OP.add)

            # ---- inclusive prefix of valid: 5 shifted adds on vector ----
            # padded tiles: cols 0:16 stay zero so shifted reads see zeros
            PAD = 16
            pfa = segpool.tile([128, PAD + NTOK], bf16, tag="pfa")
            nc.gpsimd.memset(pfa[:, 0:PAD], 0.0)
            pfb = segpool.tile([128, PAD + NTOK], bf16, tag="pfb")
            nc.gpsimd.memset(pfb[:, 0:PAD], 0.0)
            # stage s=1 (reads valid directly, two ops for the edge)
            nc.vector.tensor_copy(pfa[:, PAD:PAD + 1], valid[:, 0:1])
            nc.vector.tensor_tensor(pfa[:, PAD + 1:PAD + NTOK],
                                    valid[:, 1:NTOK], valid[:, 0:NTOK - 1],
                                    OP.add)
            cur, nxt = pfa, pfb
            for s in (2, 4, 8, 16):
                nc.vector.tensor_tensor(nxt[:, PAD:PAD + NTOK],
                                        cur[:, PAD:PAD + NTOK],
                                        cur[:, PAD - s:PAD + NTOK - s], OP.add)
                cur, nxt = nxt, cur
            pincl = cur[:, PAD:PAD + NTOK]

            # cross-partition exclusive base via strict-upper-tri matmul
            base_ps = ps_a.tile([128, 1], f32, tag="a")
            nc.tensor.matmul(base_ps[:], ltric[:], cur[:, PAD + NTOK - 1:PAD + NTOK],
                             start=True, stop=True)
            # t0 = (incl prefix + base) * valid  (invalid -> 0), then -1
            t0 = smlpool.tile([128, NTOK], f32, tag="t0")
            nc.vector.scalar_tensor_tensor(t0[:], pincl, base_ps[:],
                                           valid, OP.add, OP.mult)
            # scatter idx block a: t0 - 1 + a*NSLOT*valid (invalid stays -1);
            # block 0 is written directly as int16 by the t0-1 op itself,
            # blocks 1-3 read it back as their int16 in1 operand
            idx16 = smlpool.tile([128, 4 * NTOK], i16, tag="idx16")
            nc.vector.tensor_scalar(idx16[:, 0:NTOK], t0[:], -1.0, None, OP.add)
            for a in range(1, 4):
                nc.vector.scalar_tensor_tensor(
                    idx16[:, a * NTOK:(a + 1) * NTOK], valid,
                    float(a * NSLOT), idx16[:, 0:NTOK], OP.mult, OP.add)

            # (no gpsimd op may sit between the const DVE ops and the
            # scatter: each DVE<->scatter library swap stalls the gpsimd
            # queue ~3us, and a dummy pre-scatter also regresses -- measured)
            # keep-busy filler: a dependency-free memset that ends near the
            # time the scatter's semaphore arrives, so the engine doesn't
            # pay the deep-idle wake latency (~1.3us) on the scatter itself
            fill = smlpool.tile([128, 3072], bf16, tag="fill")
            nc.gpsimd.memset(fill[:], 0.0)

            s_pos = smlpool.tile([128, 4 * NSLOT], bf16, tag="spos")
            nc.gpsimd.local_scatter(
                s_pos[:], svals[:], idx16[:],
                channels=128, num_elems=4 * NSLOT, num_idxs=4 * NTOK)

            # ---- collapse partitions: compacted values per slot ----
            cmp_ps = ps_a.tile([128, 4], f32, tag="a")
            nc.tensor.matmul(cmp_ps[:, 0:1],
                             s_pos[:, 0:NSLOT].bitcast(f16),
                             onesb[:], start=True, stop=True)
            for a in range(1, 4):
                nc.tensor.matmul(cmp_ps[:, a:a + 1],
                                 s_pos[:, a * NSLOT:(a + 1) * NSLOT],
                                 onesb[:], start=True, stop=True)
            cmp = smlpool.tile([128, 4], f32, tag="cmpsb")
            nc.vector.tensor_copy(cmp[:], cmp_ps[:])
            mj_c = cmp[:, 0:1]
            k_c = cmp[:, 1:2]
            b_c = cmp[:, 2:3]
            sv = cmp[:, 3:4]

            # gather-critical: rows = k_c*512 + mj (direct int32 write),
            # then the indirect gather issues immediately.
            rows_i32 = smlpool.tile([128, 1], i32, tag="rows")
            nc.vector.scalar_tensor_tensor(rows_i32[:], k_c, float(L), mj_c[:],
                                           OP.mult, OP.add)

            # ---- gather only the valid embedding rows (single indirect
            # DMA: splitting into partition halves crashes the runtime) ----
            eg = cpool.tile([128, C], bf16, tag="eg")
            nc.gpsimd.indirect_dma_start(
                out=eg[:], out_offset=None, in_=ter_in[:],
                in_offset=bass.IndirectOffsetOnAxis(ap=rows_i32[:, 0:1], axis=0))

            # ---- slot-equality matrix, counts, weights (needed when eg
            # lands, ~3us after the gather issue) ----
            # k_c is k+1 (1..8; empty slots 0) -> sege/keff distinct for free
            sege = smlpool.tile([128, 1], f32, tag="sege")
            nc.vector.scalar_tensor_tensor(sege[:], k_c, 200.0, b_c,
                                           OP.mult, OP.add)
            keff = k_c

            segb_ps = ps_a.tile([128, 128], f32, tag="a")
            nc.tensor.transpose(segb_ps[:], sege[:].to_broadcast([128, 128]),
                                idtc[:])
            segT = smlpool.tile([128, 128], f32, tag="segT")
            nc.vector.tensor_copy(segT[:], segb_ps[:])
            eqm = smlpool.tile([128, 128], bf16, tag="eqm")
            nc.vector.tensor_tensor(eqm[:], sege[:].to_broadcast([128, 128]),
                                    segT[:], OP.is_equal)

            cnt_ps = ps_a.tile([128, 1], f32, tag="a")
            nc.tensor.matmul(cnt_ps[:], eqm[:], onesb[:], start=True, stop=True)
            cntm = smlpool.tile([128, 1], f32, tag="cntm")
            nc.vector.tensor_scalar(cntm[:], cnt_ps[:], 1.0, None, OP.max)
            wf = smlpool.tile([128, 1], f32, tag="wf")
            nc.vector.reciprocal(wf[:], cntm[:])
            wv = smlpool.tile([128, 1], f32, tag="wv")
            nc.vector.tensor_tensor(wv[:], wf[:], sv, OP.mult)

            # ---- post-phase helpers (only needed once logits exist) ----
            notv = smlpool.tile([128, 1], f32, tag="notv")
            nc.vector.tensor_scalar(notv[:], k_c, 0.0, None, OP.is_equal)
            mmask = cpool.tile([128, P], bf16, tag="mmask")
            nc.vector.tensor_tensor(mmask[:], b_c.to_broadcast([128, P]),
                                    iotap[:], OP.is_equal)

            # batched per-sample indicator/weight matrices
            indmat = smlpool.tile([128, SPC], f32, tag="indmat")
            nc.vector.tensor_tensor(indmat[:], keff.to_broadcast([128, SPC]),
                                    iotap[:, 1:SPC + 1], OP.is_equal)
            keff9 = smlpool.tile([128, 1], f32, tag="keff9")
            nc.vector.tensor_scalar(keff9[:], keff, 9.0, None, OP.mult)
            mask64 = smlpool.tile([128, 8 * SPC], f32, tag="mask64")
            nc.vector.tensor_tensor(mask64[:],
                                    keff9[:].to_broadcast([128, 8 * SPC]),
                                    colcc[:], OP.is_equal)
            w2sel = smlpool.tile([128, 8 * SPC], bf16, tag="w2sel")
            nc.vector.tensor_tensor(w2sel[:], mask64[:],
                                    wf[:].to_broadcast([128, 8 * SPC]), OP.mult)

            # ---- per-slot bucket sums (tpe numerators), stationary for
            # logits. Two 384-wide casts: narrower per-chunk casts cost MORE
            # (DVE fixed overhead ~190ns/op dominates at 128 cols) ----
            tpe_sb = cpool.tile([128, C], bf16, tag="tpe")
            for h in range(2):
                tp_ps = ps_c1.tile([128, 384], f32, name=f"tp{h}", tag="c1")
                for c3 in range(3):
                    ci = 3 * h + c3
                    nc.tensor.matmul(tp_ps[:, c3 * 128:(c3 + 1) * 128],
                                     eg[:, ci * 128:(ci + 1) * 128],
                                     eqm[:], start=True, stop=True)
                nc.vector.tensor_copy(tpe_sb[:, h * 384:(h + 1) * 384], tp_ps[:])

            # ---- logits [slot, 196] per sample; CE pieces ----
            exsum = accpool.tile([128, SPC], f32, tag="exsum")
            c1all = ps_c1.tile([SPC, P + 2], f32, tag="c1")

            def post_sample(k, lg):
                # scaled+clamped logits (bf16); frees the psum bank half.
                # lg is an AP (half of a paired psum tile)
                xs = xspool.tile([128, P], bf16, name=f"xs{k}", tag="xs")
                nc.vector.tensor_scalar(xs[:], lg, wf[:], CLAMP,
                                        OP.mult, OP.min)
                if KDBG and k == 0:
                    nc.sync.dma_start(dbg_xs[:], xs[:])
                # exm = [exp(xs-45) | 1 | pick]; pick = xs[slot, b_slot] via
                # a masked row-reduce (mult on vector, reduce on gpsimd --
                # both off the scalar/PE critical chain). The c1 matmul then
                # only moves P+2 columns instead of 2P+1.
                exm = expool.tile([128, P + 2], bf16, name=f"ex{k}", tag="ex")
                nc.gpsimd.memset(exm[:, P:P + 1], 1.0)
                nc.scalar.activation(exm[:, 0:P], xs[:], AF.Exp, bias=negsh[:],
                                     accum_out=exsum[:, k:k + 1])
                xsm = scrpool.tile([128, P], bf16, name=f"xsm{k}", tag="xsm")
                nc.gpsimd.tensor_tensor(xsm[:], xs[:], mmask[:], OP.mult)
                # xsm is one-hot per row (xs*mmask): bf16 accumulation exact
                with nc.allow_low_precision(reason="one-hot row reduce"):
                    nc.vector.tensor_reduce(exm[:, P + 1:P + 2], xsm[:],
                                            axis=AX.X, op=OP.add)
                # one matmul: row k of c1all += [w2^T exp | nb | w2^T pick]
                nc.tensor.matmul(c1all[0:SPC, 0:P + 2],
                                 w2sel[:, 8 * k:8 * (k + 1)],
                                 exm[:, 0:P + 2],
                                 start=(k == 0), stop=(k == SPC - 1))

            # paired matmuls: ONE matmul with a 392-col moving operand
            # computes TWO samples' logits per chunk (amortizes per-matmul
            # overhead). The first/last samples run solo so the pipeline
            # fills fast (a leading pair delays the first exp ~0.8us).
            iper = [ipeh[g][:].rearrange("p (s c q) -> p s c q", s=4, q=P)
                    for g in range(2)]

            def lg_block(g, s0, ns, name):
                t = ps_lg.tile([128, ns * P], f32, name=name, tag="lg")
                for ci in range(6):
                    nc.tensor.matmul(
                        t[:], tpe_sb[:, ci * 128:(ci + 1) * 128],
                        iper[g][:, s0:s0 + ns, ci, :],
                        start=(ci == 0), stop=(ci == 5))
                for j in range(ns):
                    post_sample(4 * g + s0 + j, t[:, j * P:(j + 1) * P])

            lg_block(0, 0, 1, "lgA")      # k0 solo (fast fill)
            lg_block(0, 1, 2, "lgB")      # k1,k2 pair
            lg_block(0, 3, 1, "lgC")      # k3 solo
            lg_block(1, 0, 2, "lgD")      # k4,k5 pair
            lg_block(1, 2, 2, "lgE")      # k6,k7 pair

            # ---- final reduction ----
            # per-sample scalars now live on partitions 0..7 of c1all
            tbs8 = smlpool.tile([SPC, 1], f32, tag="tbs8")
            nc.vector.tensor_scalar(tbs8[:], c1all[0:SPC, P:P + 1],
                                    -EM45, float(P) * EM45, OP.mult, OP.add)
            # tbtf = (LN196-SHIFT) * (P - tb), constants folded in
            tbtf = smlpool.tile([SPC, 1], f32, tag="tbtf")
            nc.vector.tensor_scalar(tbtf[:], c1all[0:SPC, P:P + 1],
                                    -(LN196 - SHIFT), float(P) * (LN196 - SHIFT),
                                    OP.mult, OP.add)
            nc.vector.tensor_scalar(lnpack[0:SPC, 1:P + 1], c1all[0:SPC, 0:P],
                                    tbs8[:], None, OP.add)
            # dred8 accumulated directly by the c1 matmul's pick column
            dred8 = c1all[0:SPC, P + 1:P + 2]
            # orientation2 row sums: select own-sample column of exsum
            r2sel = smlpool.tile([128, SPC], f32, tag="r2sel")
            nc.vector.tensor_tensor(r2sel[:], exsum[:], indmat[:], OP.mult)
            r2comb = smlpool.tile([128, 1], f32, tag="r2comb")
            nc.vector.tensor_reduce(r2comb[:], r2sel[:], axis=AX.X, op=OP.add)
            nc.vector.tensor_tensor(lnpack[:, 0:1], r2comb[:], notv[:], OP.add)

            # ONE packed Ln: lnout[:,0] = ln(r2safe); rows 0:8 accum =
            # ln1a + lnr2 (col-0 contamination subtracted below)
            lnout = smlpool.tile([128, P + 1], f32, tag="lnout")
            lnacc = smlpool.tile([128, 1], f32, tag="lnacc")
            nc.scalar.activation(lnout[:], lnpack[:], AF.Ln, accum_out=lnacc[:])
            lnr2 = lnout[:, 0:1]

            # e8[k] = ln1a + (ln196-45)*(P-tb) - 2*dred ; shift const later.
            # e8p (everything but ln1a) is ready before the Lns finish.
            e8p = smlpool.tile([SPC, 1], f32, tag="e8p")
            nc.vector.scalar_tensor_tensor(e8p[:], dred8[:], -2.0, tbtf[:],
                                           OP.mult, OP.add)
            cA = smlpool.tile([128, 1], f32, tag="cA")
            nc.vector.tensor_tensor(cA[:], lnr2[:], wv[:], OP.mult)
            e8b = smlpool.tile([SPC, 1], f32, tag="e8b")
            nc.vector.scalar_tensor_tensor(e8b[:], lnacc[0:SPC, 0:1],
                                           lnr2[0:SPC, 0:1], e8p[:],
                                           OP.subtract, OP.add)

            fin_ps = ps_a.tile([1, 2], f32, tag="a")
            nc.tensor.matmul(fin_ps[:, 0:1], cA[:], onesf[:],
                             start=True, stop=True)
            nc.tensor.matmul(fin_ps[:, 1:2], e8b[:], onesf[0:SPC, 0:1],
                             start=True, stop=True)
            finsb = smlpool.tile([1, 2], f32, tag="finsb")
            nc.vector.tensor_copy(finsb[:], fin_ps[0:1, :])
            res = smlpool.tile([1, 1], f32, tag="res")
            nc.vector.scalar_tensor_tensor(res[:], finsb[:, 0:1],
                                           2.0 * SHIFT * P * SPC,
                                           finsb[:, 1:2], OP.add, OP.add)
            nc.sync.dma_start(out_ext[:], res[:])
            if KDBG:
                nc.sync.dma_start(dbg_cmp[:], cmp[:])
                dbgw = smlpool.tile([128, 8], f32, tag="dbgw")
                nc.vector.tensor_copy(dbgw[:, 0:1], cntm[:])
                nc.vector.tensor_copy(dbgw[:, 1:2], wf[:])
                nc.vector.tensor_copy(dbgw[:, 2:3], wv[:])
                nc.vector.tensor_copy(dbgw[:, 3:4], notv[:])
                nc.vector.tensor_copy(dbgw[:, 4:5], sege[:])
                nc.vector.tensor_copy(dbgw[:, 5:6], keff[:])
                nc.vector.tensor_copy(dbgw[:, 6:7], rows_i32[:])
                nc.vector.tensor_copy(dbgw[:, 7:8], sv)
                nc.sync.dma_start(dbg_w[:], dbgw[:])
                nc.sync.dma_start(dbg_eq[:], eqm[:])
                nc.sync.dma_start(dbg_eg[:], eg[:])
                dbgc1 = smlpool.tile([SPC, P + 2], f32, tag="dbgc1")
                nc.vector.tensor_copy(dbgc1[:], c1all[0:SPC, :])
                nc.sync.dma_start(dbg_c1[:], dbgc1[:])
                nc.sync.dma_start(dbg_ex[:], exsum[:])
                dbgf = smlpool.tile([128, 8], f32, tag="dbgf")
                nc.vector.tensor_copy(dbgf[:, 0:1], r2comb[:])
                nc.vector.tensor_copy(dbgf[:, 1:2], lnpack[:, 0:1])
                nc.vector.tensor_copy(dbgf[:, 2:3], lnr2)
                nc.vector.tensor_copy(dbgf[:, 3:4], cA[:])
                nc.vector.tensor_copy(dbgf[0:SPC, 4:5], lnacc[0:SPC, 0:1])
                nc.vector.tensor_copy(dbgf[0:SPC, 5:6], dred8[:])
                nc.vector.tensor_copy(dbgf[0:SPC, 6:7], tbtf[:])
                nc.vector.tensor_copy(dbgf[0:1, 7:8], res[:])
                nc.sync.dma_start(dbg_fin[:], dbgf[:])

    nc.compile()
    _NC_CACHE = nc
    return nc


def _stage_core(te, ipe, bbox, am, c):
    """Build the in_map for core c from full inputs."""
    import ml_dtypes
    bf = ml_dtypes.bfloat16
    sl = slice(c * SPC, (c + 1) * SPC)
    ter = te[sl].reshape(SPC * L, C).astype(bf)
    ipet = (np.ascontiguousarray(ipe[sl]).transpose(0, 2, 1)   # [SPC, 768, 196]
            .reshape(SPC, 6, 128, P).transpose(2, 0, 1, 3)
            .reshape(128, SPC * 6 * P)).astype(bf)
    bbq = (bbox[sl].astype(np.int32).reshape(SPC, 128, TPD, 4)
           .transpose(1, 0, 2, 3).reshape(128, SPC * 16))
    mq = (am[sl].astype(np.float32).reshape(SPC, 128, TPD)
          .transpose(1, 0, 2).reshape(128, NTOK))
    return {
        "ter": np.ascontiguousarray(ter),
        "ipes": np.ascontiguousarray(ipet),
        "bboxq": np.ascontiguousarray(bbq),
        "maskq": np.ascontiguousarray(mq),
    }


def _check_capacity(bbox, am):
    d = (bbox.astype(np.int64) // 72)
    val = ((d[..., 0] == d[..., 2]) & (d[..., 1] == d[..., 3])
           & (am != 0))
    per_core = val.reshape(NCORES, SPC * L).sum(axis=1)
    assert per_core.max() <= NSLOT, (
        f"valid-token count {per_core.max()} exceeds slot capacity {NSLOT}")


def _install_profile_hook():
    """Wire the NTFF profile hook (the image's antenv lacks axon_hooks)."""
    import sys
    import types
    try:
        import antenv.axon_hooks  # noqa: F401
        return
    except ImportError:
        pass
    import antenv
    mod = types.ModuleType("antenv.axon_hooks")
    holder = {}
    mod.set_axon_ntff_profile_hook = lambda h: holder.__setitem__("h", h)
    mod.get_axon_ntff_profile_hook = lambda: holder.get("h")
    sys.modules["antenv.axon_hooks"] = mod
    antenv.axon_hooks = mod
    from trn_agent_boot.trn_boot import _ntff_profile_via_ctypes
    mod.set_axon_ntff_profile_hook(
        _ntff_profile_via_ctypes("/opt/axon/libaxon_pjrt.so"))
    import concourse.bass_utils as bu
    bu.upload_artifacts = lambda tmpdir: f"local:{tmpdir}"


def _run(inputs, trace=False, trace_kwargs=None):
    from concourse.bass_utils import run_bass_kernel_spmd
    if trace:
        _install_profile_hook()
    te = np.asarray(inputs["text_embeds"], dtype=np.float32)
    ipe = np.asarray(inputs["image_patch_embedding"], dtype=np.float32)
    bbox = np.asarray(inputs["bbox"])
    am = np.asarray(inputs["attention_mask"])
    _check_capacity(bbox, am)
    nc = _build_nc()
    in_maps = [_stage_core(te, ipe, bbox, am, c) for c in range(NCORES)]
    kw = {}
    if trace:
        kw = dict(trace=True, trace_kwargs=trace_kwargs or {})
    res = run_bass_kernel_spmd(nc, in_maps, core_ids=list(range(NCORES)), **kw)
    total = sum(float(res.results[i]["out"][0, 0]) for i in range(NCORES))
    loss = total / (2.0 * B * P)
    return np.asarray(loss, dtype=np.float32), res


def kernel(**inputs) -> np.ndarray:
    try:
        loss, _ = _run(inputs, trace=False)
    except Exception:
        # one retry: a previously wedged device recovers after a failed call
        loss, _ = _run(inputs, trace=False)
    return loss


# revision 52
# speedup vs baseline: 1.0546x; 1.0546x over previous
"""Trainium2 Bass kernel for nn_AET_34737695490187 (histogram_binning).

Math (per sample):
  d = bbox // 72; label = y0*14+x0 where (x0==x1 & y0==y1 & mask) else invalid
  segment-sum text_embeds into 196 patch buckets -> sums, counts
  tpe = sums / max(counts, 1)
  logits1 = ipe @ tpe^T (per sample); logits2 = logits1^T
  loss = mean of CE(logits, diag) both ways / 2

Key observation: validity requires x0==x1 & y0==y1 & mask, true for ~1/392
of tokens (~10-20 per 8-sample shard). Streaming all of text_embeds (and
the dense 512x196 scatter matmul) is ~99% wasted HBM traffic / PE columns.
This kernel instead:

  1. computes validity + bucket labels on-device from bbox (tiny),
  2. compacts valid tokens into <=128 global slots: per-partition
     local_scatter into slot columns, then a ones-matmul collapses
     partitions (each slot column has exactly one contributor),
  3. indirect-DMA-gathers only those embedding rows from DRAM,
  4. forms per-slot bucket sums with a slot-equality matrix EQ
     (duplicate buckets handled exactly: every slot of a bucket carries
     the full bucket sum and weight w=1/count, so per-bucket sums become
     w-weighted slot sums),
  5. computes logits [slot, 196] per sample with the bucket sums
     stationary (one 128-wide LDWEIGHTS per c-chunk, shared by all
     8 samples via separate psum banks),
  6. CE via zero structure: empty buckets contribute exp(0)=1
     (orientation 1) resp. ln(196) (orientation 2), so only slot
     rows/columns are exponentiated. No per-row max: logits are clamped
     at 83 (validated on data: ~1.5e-3 rel err vs 2e-2 tolerance).
  7. all Exps run before all Lns (one activation-table reload); the two
     final Lns are packed into ONE activation over [r2safe | r1mat/ones]
     with the accumulator contamination subtracted by a single STT.

Scheduling (measured on HW, 47.9us -> ~38us):
  - every small constant (triangular/identity masks, iota ramps,
    slot-select patterns) is generated on-device with gpsimd
    iota/affine_select/memset -- DMA'd constants queue behind the 2.4MB
    ipe stream and land 10+us late (per-ring FIFO, cold rings ~3.5us).
  - bbox/mask go FIRST on the warm sync ring, preceded by a tiny warm-up
    transfer (one DMA engine wakes ~2.4us late; the semaphore waits on
    the slowest of all 16 engines).
  - the valid-prefix-scan is 5 shifted adds on the vector engine (padded
    zero region) instead of PE transpose -> tri matmul -> transpose.
  - the row id for the gather is scattered as one fp16 value (4m+j-512)
    so rows = 512*k + mj needs a single int32-writing STT; empty slots
    dedupe to row 0 (masked downstream by w2sel/indmat).
  - logits run block-major (solo samples on the flanks, 392-col paired
    matmuls in the middle) with each sample's post chain (xs -> exp ->
    pick) issued immediately, pipelining V/S/G engines against the
    PE-bound logits+c1 matmul stream; the c1 matmul moves only P+2
    columns (the diag-picked logit is pre-reduced into one column).
  - no gpsimd DVE op may interleave with local_scatter/indirect-DMA:
    each gpsimd library swap stalls its queue ~3us.

Sharding: pure data parallel, 8 samples per core on 8 NeuronCores.
"""

import os

import numpy as np

KDBG = bool(os.environ.get("KDBG"))

B, L, C, P = 64, 512, 768, 196
NCORES = 8
SPC = B // NCORES          # samples per core
TPD = 4                    # tokens per partition-col block (512 = 128*4)
NTOK = SPC * TPD           # 32 token cols per partition
NSLOT = 128                # global compacted-slot capacity per core
CLAMP = 83.0
SHIFT = 45.0                # exp/ln range shift: ACT Ln breaks above ~2^64
EM45 = float(np.exp(-45.0))
LN196 = float(np.log(196.0))

_NC_CACHE = None


def _build_nc():
    global _NC_CACHE
    if _NC_CACHE is not None:
        return _NC_CACHE

    import concourse.bass as bass
    import concourse.mybir as mybir
    from concourse import bacc, tile

    f32 = mybir.dt.float32
    bf16 = mybir.dt.bfloat16
    i32 = mybir.dt.int32
    i16 = mybir.dt.int16
    AF = mybir.ActivationFunctionType
    OP = mybir.AluOpType
    AX = mybir.AxisListType
    PSUM = bass.MemorySpace.PSUM

    nc = bacc.Bacc(target_bir_lowering=False, debug=False)

    ter_in = nc.dram_tensor("ter", [SPC * L, C], bf16, kind="ExternalInput")
    ipe_in = nc.dram_tensor("ipes", [128, SPC * 6 * P], bf16, kind="ExternalInput")
    bbox_in = nc.dram_tensor("bboxq", [128, SPC * 16], i32, kind="ExternalInput")
    mask_in = nc.dram_tensor("maskq", [128, NTOK], f32, kind="ExternalInput")
    out_ext = nc.dram_tensor("out", [1, 1], f32, kind="ExternalOutput")
    if KDBG:
        dbg_cmp = nc.dram_tensor("dbg_cmp", [128, 4], f32, kind="ExternalOutput")
        dbg_w = nc.dram_tensor("dbg_w", [128, 8], f32, kind="ExternalOutput")
        dbg_eq = nc.dram_tensor("dbg_eq", [128, 128], bf16, kind="ExternalOutput")
        dbg_xs = nc.dram_tensor("dbg_xs", [128, P], bf16, kind="ExternalOutput")
        dbg_eg = nc.dram_tensor("dbg_eg", [128, C], bf16, kind="ExternalOutput")
        dbg_c1 = nc.dram_tensor("dbg_c1", [SPC, P + 2], f32,
                                kind="ExternalOutput")
        dbg_ex = nc.dram_tensor("dbg_ex", [128, SPC], f32, kind="ExternalOutput")
        dbg_fin = nc.dram_tensor("dbg_fin", [128, 8], f32, kind="ExternalOutput")
    with tile.TileContext(nc) as tc:
        with (
            tc.tile_pool(name="const", bufs=1) as cpool,
            tc.tile_pool(name="ipe", bufs=SPC) as ipool,
            tc.tile_pool(name="seg", bufs=1) as segpool,
            tc.tile_pool(name="sml", bufs=1) as smlpool,
            tc.tile_pool(name="xs", bufs=3) as xspool,
            tc.tile_pool(name="ex", bufs=3) as expool,
            tc.tile_pool(name="scr", bufs=4) as scrpool,
            tc.tile_pool(name="acc", bufs=1) as accpool,
            tc.tile_pool(name="ps_a", bufs=1, space=PSUM) as ps_a,
            tc.tile_pool(name="ps_lg", bufs=2, space=PSUM) as ps_lg,
            tc.tile_pool(name="ps_c1", bufs=2, space=PSUM) as ps_c1,
        ):
            # ---- DMA in: bbox/mask FIRST on the sync ring, ahead of the ipe
            # stream. The sync ring is warm (template uses it) and packets
            # drain FIFO per ring, so these land in ~2us; on any other ring
            # their packets interleave 1:1 with the 1us-sized ipe packets
            # and take 10us+ (measured). A tiny warm-up transfer goes first:
            # some DMA engines take ~2.4us to wake from their doorbell, and
            # bbox's semaphore waits on the slowest of all 16. ----
            dwarm = cpool.tile([128, 1], i32, tag="dwarm")
            nc.sync.dma_start(dwarm[:], bbox_in[:, 0:1])
            bboxa = cpool.tile([128, SPC * 16], i32, tag="bboxa")
            nc.sync.dma_start(bboxa[:], bbox_in[:])
            maska = cpool.tile([128, NTOK], f32, tag="maska")
            nc.sync.dma_start(maska[:], mask_in[:])

            # big ipe stream: two halves on the sync ring, right behind
            ipeh = []
            for g in range(2):
                t = ipool.tile([128, 4 * 6 * P], bf16, name=f"ipeh{g}", tag="ipet")
                nc.sync.dma_start(t[:], ipe_in[:, g * 4 * 6 * P:(g + 1) * 4 * 6 * P])
                ipeh.append(t)
            ipet = [ipeh[k // 4][:, (k % 4) * 6 * P:(k % 4 + 1) * 6 * P]
                    for k in range(SPC)]

            # ---- on-device constants (gpsimd; no DMA) ----
            onesc = cpool.tile([128, 2], f32, tag="onesc")
            nc.gpsimd.memset(onesc[:], 1.0)
            onesf = onesc[:, 0:1]
            onesb = cpool.tile([128, 1], bf16, tag="onesb")
            nc.gpsimd.memset(onesb[:], 1.0)
            negsh = cpool.tile([128, 1], f32, tag="negsh")
            nc.gpsimd.memset(negsh[:], -SHIFT)
            kconst = cpool.tile([128, 1], f32, tag="kconst")
            nc.gpsimd.memset(kconst[:], 2.0 * SHIFT * P * SPC / 128.0)

            # PE warmup source, needed first
            wsrc = cpool.tile([128, 128], bf16, tag="wsrc")
            nc.gpsimd.memset(wsrc[:], 0.0)

            # packed-Ln staging area: col 0 <- r2safe, cols 1:P+1 rows 0:8
            # <- r1mat; everything else stays 1.0 (ln(1)=0, accum-neutral)
            lnpack = cpool.tile([128, P + 1], f32, tag="lnpack")
            nc.gpsimd.memset(lnpack[:], 1.0)

            # strict-lower-as-columns mask: ltri[r, c] = 1 iff r < c
            ltric = cpool.tile([128, 128], bf16, tag="ltri")
            nc.gpsimd.memset(ltric[:], 1.0)
            nc.gpsimd.affine_select(
                out=ltric[:], in_=ltric[:], compare_op=OP.is_gt, fill=0.0,
                base=0, pattern=[[1, 128]], channel_multiplier=-1)

            # svals layout: [mj (fp16 bits) | k+1 | bucket | valid], 16-bit.
            # mj = 4m + j - 512 in fp16 (exact: |mj| <= 512 < 2048); the
            # token's ter row is then k_c*512 + mj with ter holding the 8
            # samples back-to-back (no zero block). Empty slots zero-fill
            # to mj=0, k_c=0 -> row 0: a harmless duplicate gather whose
            # slots are masked out downstream (w2sel/indmat are 0 there).
            f16 = mybir.dt.float16
            svals = smlpool.tile([128, 4 * NTOK], bf16, tag="svals")
            nc.gpsimd.iota(svals[:, 0:NTOK].bitcast(f16), [[0, SPC], [1, TPD]],
                           base=-512, channel_multiplier=4,
                           allow_small_or_imprecise_dtypes=True)
            nc.gpsimd.iota(svals[:, NTOK:2 * NTOK], [[1, SPC], [0, TPD]],
                           base=1, channel_multiplier=0,
                           allow_small_or_imprecise_dtypes=True)

            # fp32 identity (PE transpose helper for sege)
            idtc = cpool.tile([128, 128], f32, tag="idt")
            nc.gpsimd.memset(idtc[:], 1.0)
            nc.gpsimd.affine_select(
                out=idtc[:], in_=idtc[:], compare_op=OP.is_equal, fill=0.0,
                base=0, pattern=[[-1, 128]], channel_multiplier=1)

            # iotap[p, i] = i  (fp32 ramp 0..P-1)
            iotap = cpool.tile([128, P], f32, tag="iotap")
            nc.gpsimd.iota(iotap[:], [[1, P]], channel_multiplier=0,
                           allow_small_or_imprecise_dtypes=True)

            # colc[p, 8k+j] = 9(k+1) if j == k else -1  (w2sel selector)
            colcc = cpool.tile([128, 8 * SPC], f32, tag="colc")
            nc.gpsimd.iota(colcc[:], [[9, SPC], [0, 8]], base=9,
                           channel_multiplier=0,
                           allow_small_or_imprecise_dtypes=True)
            nc.gpsimd.affine_select(
                out=colcc[:], in_=colcc[:], compare_op=OP.is_equal, fill=-1.0,
                base=0, pattern=[[-1, SPC], [1, 8]], channel_multiplier=0)

            # warm the Exp activation table off the critical path. (Warming
            # Ln too does NOT help: the compiler reloads the table on every
            # Exp<->Ln switch, so an early Ln just inserts an extra 1.28us
            # Exp reload before the post-phase Exps -- measured.)
            wdum = smlpool.tile([1, 1], f32, tag="wdum")
            nc.scalar.activation(wdum[:], onesc[0:1, 0:1], AF.Exp)

            # PE HAM warmup: keep TensorE awake until the first real matmul
            wps = ps_a.tile([128, 128], f32, tag="a")
            NWARM = 14
            for wi in range(NWARM):
                nc.tensor.matmul(wps[:], wsrc[:], wsrc[:],
                                 start=(wi == 0), stop=(wi == NWARM - 1))

            # ---- seg chain: validity + bucket per token ----
            # d = bbox // 72 via (bbox * 1821) >> 17 (dual-op arith+bitwise
            # is rejected by the BIR verifier, so two instructions)
            mtile = segpool.tile([128, SPC * 16], i32, tag="mtile")
            nc.vector.tensor_scalar(mtile[:], bboxa[:], 1821, None, OP.mult)
            dtile = segpool.tile([128, SPC * 16], i32, tag="dtile")
            nc.vector.tensor_scalar(dtile[:], mtile[:], 17, None,
                                    OP.arith_shift_right)
            d3 = dtile[:].rearrange("p (t c) -> p t c", c=4)

            eqs = segpool.tile([128, 2 * NTOK], f32, tag="eqs")
            eqs3 = eqs[:].rearrange("p (t c) -> p t c", c=2)
            nc.vector.tensor_tensor(eqs3, d3[:, :, 0:2], d3[:, :, 2:4], OP.is_equal)

            v1 = segpool.tile([128, NTOK], f32, tag="v1")
            v1v = v1[:].rearrange("p (t o) -> p t o", o=1)
            nc.vector.tensor_tensor(v1v, eqs3[:, :, 0:1], eqs3[:, :, 1:2], OP.mult)
            valid = svals[:, 3 * NTOK:4 * NTOK]
            nc.vector.tensor_tensor(valid, v1[:], maska[:], OP.mult)

            buckv = (svals[:, 2 * NTOK:3 * NTOK]
                     .rearrange("p (t o) -> p t o", o=1))
            nc.vector.scalar_tensor_tensor(
                buckv, d3[:, :, 1:2], 14.0, d3[:, :, 0:1], OP.mult, OP.add)

            # ---- inclusive prefix of valid: 5 shifted adds on vector ----
            # padded tiles: cols 0:16 stay zero so shifted reads see zeros
            PAD = 16
            pfa = segpool.tile([128, PAD + NTOK], bf16, tag="pfa")
            nc.gpsimd.memset(pfa[:, 0:PAD], 0.0)
            pfb = segpool.tile([128, PAD + NTOK], bf16, tag="pfb")
            nc.gpsimd.memset(pfb[:, 0:PAD], 0.0)
            # stage s=1 (reads valid directly, two ops for the edge)
            nc.vector.tensor_copy(pfa[:, PAD:PAD + 1], valid[:, 0:1])
            nc.vector.tensor_tensor(pfa[:, PAD + 1:PAD + NTOK],
                                    valid[:, 1:NTOK], valid[:, 0:NTOK - 1],
                                    OP.add)
            cur, nxt = pfa, pfb
            for s in (2, 4, 8, 16):
                nc.vector.tensor_tensor(nxt[:, PAD:PAD + NTOK],
                                        cur[:, PAD:PAD + NTOK],
                                        cur[:, PAD - s:PAD + NTOK - s], OP.add)
                cur, nxt = nxt, cur
            pincl = cur[:, PAD:PAD + NTOK]

            # cross-partition exclusive base via strict-upper-tri matmul
            base_ps = ps_a.tile([128, 1], f32, tag="a")
            nc.tensor.matmul(base_ps[:], ltric[:], cur[:, PAD + NTOK - 1:PAD + NTOK],
                             start=True, stop=True)
            # t0 = (incl prefix + base) * valid  (invalid -> 0), then -1
            t0 = smlpool.tile([128, NTOK], f32, tag="t0")
            nc.vector.scalar_tensor_tensor(t0[:], pincl, base_ps[:],
                                           valid, OP.add, OP.mult)
            # scatter idx block a: t0 - 1 + a*NSLOT*valid (invalid stays -1);
            # block 0 is written directly as int16 by the t0-1 op itself,
            # blocks 1-3 read it back as their int16 in1 operand
            idx16 = smlpool.tile([128, 4 * NTOK], i16, tag="idx16")
            nc.vector.tensor_scalar(idx16[:, 0:NTOK], t0[:], -1.0, None, OP.add)
            for a in range(1, 4):
                nc.vector.scalar_tensor_tensor(
                    idx16[:, a * NTOK:(a + 1) * NTOK], valid,
                    float(a * NSLOT), idx16[:, 0:NTOK], OP.mult, OP.add)

            # (no gpsimd op may sit between the const DVE ops and the
            # scatter: each DVE<->scatter library swap stalls the gpsimd
            # queue ~3us, and a dummy pre-scatter also regresses -- measured)
            s_pos = smlpool.tile([128, 4 * NSLOT], bf16, tag="spos")
            nc.gpsimd.local_scatter(
                s_pos[:], svals[:], idx16[:],
                channels=128, num_elems=4 * NSLOT, num_idxs=4 * NTOK)

            # ---- collapse partitions: compacted values per slot ----
            cmp_ps = ps_a.tile([128, 4], f32, tag="a")
            nc.tensor.matmul(cmp_ps[:, 0:1],
                             s_pos[:, 0:NSLOT].bitcast(f16),
                             onesb[:], start=True, stop=True)
            for a in range(1, 4):
                nc.tensor.matmul(cmp_ps[:, a:a + 1],
                                 s_pos[:, a * NSLOT:(a + 1) * NSLOT],
                                 onesb[:], start=True, stop=True)
            cmp = smlpool.tile([128, 4], f32, tag="cmpsb")
            nc.vector.tensor_copy(cmp[:], cmp_ps[:])
            mj_c = cmp[:, 0:1]
            k_c = cmp[:, 1:2]
            b_c = cmp[:, 2:3]
            sv = cmp[:, 3:4]

            # gather-critical: rows = k_c*512 + mj (direct int32 write),
            # then the indirect gather issues immediately.
            rows_i32 = smlpool.tile([128, 1], i32, tag="rows")
            nc.vector.scalar_tensor_tensor(rows_i32[:], k_c, float(L), mj_c[:],
                                           OP.mult, OP.add)

            # ---- gather only the valid embedding rows (single indirect
            # DMA: splitting into partition halves crashes the runtime) ----
            eg = cpool.tile([128, C], bf16, tag="eg")
            nc.gpsimd.indirect_dma_start(
                out=eg[:], out_offset=None, in_=ter_in[:],
                in_offset=bass.IndirectOffsetOnAxis(ap=rows_i32[:, 0:1], axis=0))

            # ---- slot-equality matrix, counts, weights (needed when eg
            # lands, ~3us after the gather issue) ----
            # k_c is k+1 (1..8; empty slots 0) -> sege/keff distinct for free
            sege = smlpool.tile([128, 1], f32, tag="sege")
            nc.vector.scalar_tensor_tensor(sege[:], k_c, 200.0, b_c,
                                           OP.mult, OP.add)
            keff = k_c

            segb_ps = ps_a.tile([128, 128], f32, tag="a")
            nc.tensor.transpose(segb_ps[:], sege[:].to_broadcast([128, 128]),
                                idtc[:])
            segT = smlpool.tile([128, 128], f32, tag="segT")
            nc.vector.tensor_copy(segT[:], segb_ps[:])
            eqm = smlpool.tile([128, 128], bf16, tag="eqm")
            nc.vector.tensor_tensor(eqm[:], sege[:].to_broadcast([128, 128]),
                                    segT[:], OP.is_equal)

            cnt_ps = ps_a.tile([128, 1], f32, tag="a")
            nc.tensor.matmul(cnt_ps[:], eqm[:], onesb[:], start=True, stop=True)
            cntm = smlpool.tile([128, 1], f32, tag="cntm")
            nc.vector.tensor_scalar(cntm[:], cnt_ps[:], 1.0, None, OP.max)
            wf = smlpool.tile([128, 1], f32, tag="wf")
            nc.vector.reciprocal(wf[:], cntm[:])
            wv = smlpool.tile([128, 1], f32, tag="wv")
            nc.vector.tensor_tensor(wv[:], wf[:], sv, OP.mult)

            # ---- post-phase helpers (only needed once logits exist) ----
            notv = smlpool.tile([128, 1], f32, tag="notv")
            nc.vector.tensor_scalar(notv[:], k_c, 0.0, None, OP.is_equal)
            mmask = cpool.tile([128, P], bf16, tag="mmask")
            nc.vector.tensor_tensor(mmask[:], b_c.to_broadcast([128, P]),
                                    iotap[:], OP.is_equal)

            # batched per-sample indicator/weight matrices
            indmat = smlpool.tile([128, SPC], f32, tag="indmat")
            nc.vector.tensor_tensor(indmat[:], keff.to_broadcast([128, SPC]),
                                    iotap[:, 1:SPC + 1], OP.is_equal)
            keff9 = smlpool.tile([128, 1], f32, tag="keff9")
            nc.vector.tensor_scalar(keff9[:], keff, 9.0, None, OP.mult)
            mask64 = smlpool.tile([128, 8 * SPC], f32, tag="mask64")
            nc.vector.tensor_tensor(mask64[:],
                                    keff9[:].to_broadcast([128, 8 * SPC]),
                                    colcc[:], OP.is_equal)
            w2sel = smlpool.tile([128, 8 * SPC], bf16, tag="w2sel")
            nc.vector.tensor_tensor(w2sel[:], mask64[:],
                                    wf[:].to_broadcast([128, 8 * SPC]), OP.mult)

            # ---- per-slot bucket sums (tpe numerators), stationary for
            # logits. Two 384-wide casts: narrower per-chunk casts cost MORE
            # (DVE fixed overhead ~190ns/op dominates at 128 cols) ----
            tpe_sb = cpool.tile([128, C], bf16, tag="tpe")
            for h in range(2):
                tp_ps = ps_c1.tile([128, 384], f32, name=f"tp{h}", tag="c1")
                for c3 in range(3):
                    ci = 3 * h + c3
                    nc.tensor.matmul(tp_ps[:, c3 * 128:(c3 + 1) * 128],
                                     eg[:, ci * 128:(ci + 1) * 128],
                                     eqm[:], start=True, stop=True)
                nc.vector.tensor_copy(tpe_sb[:, h * 384:(h + 1) * 384], tp_ps[:])

            # ---- logits [slot, 196] per sample; CE pieces ----
            exsum = accpool.tile([128, SPC], f32, tag="exsum")
            c1all = ps_c1.tile([SPC, P + 2], f32, tag="c1")

            def post_sample(k, lg):
                # scaled+clamped logits (bf16); frees the psum bank half.
                # lg is an AP (half of a paired psum tile)
                xs = xspool.tile([128, P], bf16, name=f"xs{k}", tag="xs")
                nc.vector.tensor_scalar(xs[:], lg, wf[:], CLAMP,
                                        OP.mult, OP.min)
                if KDBG and k == 0:
                    nc.sync.dma_start(dbg_xs[:], xs[:])
                # exm = [exp(xs-45) | 1 | pick]; pick = xs[slot, b_slot] via
                # a masked row-reduce (mult on vector, reduce on gpsimd --
                # both off the scalar/PE critical chain). The c1 matmul then
                # only moves P+2 columns instead of 2P+1.
                exm = expool.tile([128, P + 2], bf16, name=f"ex{k}", tag="ex")
                nc.gpsimd.memset(exm[:, P:P + 1], 1.0)
                if k < SPC - 1:
                    nc.scalar.activation(exm[:, 0:P], xs[:], AF.Exp,
                                         bias=negsh[:],
                                         accum_out=exsum[:, k:k + 1])
                else:
                    # last sample: row-sum on vector instead of the scalar
                    # accumulator read, so the 1.28us Ln table load starts
                    # ~280ns earlier on the scalar queue
                    nc.scalar.activation(exm[:, 0:P], xs[:], AF.Exp,
                                         bias=negsh[:])
                    nc.vector.tensor_reduce(exsum[:, k:k + 1], exm[:, 0:P],
                                            axis=AX.X, op=OP.add)
                xsm = scrpool.tile([128, P], bf16, name=f"xsm{k}", tag="xsm")
                nc.gpsimd.tensor_tensor(xsm[:], xs[:], mmask[:], OP.mult)
                # xsm is one-hot per row (xs*mmask): bf16 accumulation exact
                with nc.allow_low_precision(reason="one-hot row reduce"):
                    nc.vector.tensor_reduce(exm[:, P + 1:P + 2], xsm[:],
                                            axis=AX.X, op=OP.add)
                # one matmul: row k of c1all += [w2^T exp | nb | w2^T pick]
                nc.tensor.matmul(c1all[0:SPC, 0:P + 2],
                                 w2sel[:, 8 * k:8 * (k + 1)],
                                 exm[:, 0:P + 2],
                                 start=(k == 0), stop=(k == SPC - 1))

            # paired matmuls: ONE matmul with a 392-col moving operand
            # computes TWO samples' logits per chunk (amortizes per-matmul
            # overhead). The first/last samples run solo so the pipeline
            # fills fast (a leading pair delays the first exp ~0.8us).
            iper = [ipeh[g][:].rearrange("p (s c q) -> p s c q", s=4, q=P)
                    for g in range(2)]

            def lg_block(g, s0, ns, name):
                t = ps_lg.tile([128, ns * P], f32, name=name, tag="lg")
                for ci in range(6):
                    nc.tensor.matmul(
                        t[:], tpe_sb[:, ci * 128:(ci + 1) * 128],
                        iper[g][:, s0:s0 + ns, ci, :],
                        start=(ci == 0), stop=(ci == 5))
                for j in range(ns):
                    post_sample(4 * g + s0 + j, t[:, j * P:(j + 1) * P])

            lg_block(0, 0, 1, "lgA")      # k0 solo (fast fill)
            lg_block(0, 1, 2, "lgB")      # k1,k2 pair
            lg_block(0, 3, 1, "lgC")      # k3 solo
            lg_block(1, 0, 2, "lgD")      # k4,k5 pair
            lg_block(1, 2, 2, "lgE")      # k6,k7 pair

            # ---- final reduction ----
            # per-sample scalars now live on partitions 0..7 of c1all
            tbs8 = smlpool.tile([SPC, 1], f32, tag="tbs8")
            nc.vector.tensor_scalar(tbs8[:], c1all[0:SPC, P:P + 1],
                                    -EM45, float(P) * EM45, OP.mult, OP.add)
            # tbtf = (LN196-SHIFT) * (P - tb), constants folded in
            tbtf = smlpool.tile([SPC, 1], f32, tag="tbtf")
            nc.vector.tensor_scalar(tbtf[:], c1all[0:SPC, P:P + 1],
                                    -(LN196 - SHIFT), float(P) * (LN196 - SHIFT),
                                    OP.mult, OP.add)
            nc.vector.tensor_scalar(lnpack[0:SPC, 1:P + 1], c1all[0:SPC, 0:P],
                                    tbs8[:], None, OP.add)
            # dred8 accumulated directly by the c1 matmul's pick column
            dred8 = c1all[0:SPC, P + 1:P + 2]
            # orientation2 row sums: select own-sample column of exsum
            r2sel = smlpool.tile([128, SPC], f32, tag="r2sel")
            nc.vector.tensor_tensor(r2sel[:], exsum[:], indmat[:], OP.mult)
            r2comb = smlpool.tile([128, 1], f32, tag="r2comb")
            nc.vector.tensor_reduce(r2comb[:], r2sel[:], axis=AX.X, op=OP.add)
            nc.vector.tensor_tensor(lnpack[:, 0:1], r2comb[:], notv[:], OP.add)

            # ONE packed Ln: lnout[:,0] = ln(r2safe); rows 0:8 accum =
            # ln1a + lnr2 (col-0 contamination subtracted below)
            lnout = smlpool.tile([128, P + 1], f32, tag="lnout")
            lnacc = smlpool.tile([128, 1], f32, tag="lnacc")
            nc.scalar.activation(lnout[:], lnpack[:], AF.Ln, accum_out=lnacc[:])
            lnr2 = lnout[:, 0:1]

            # e8[k] = ln1a + (ln196-45)*(P-tb) - 2*dred ; shift const later.
            # e8p (everything but ln1a) is ready before the Lns finish.
            e8p = smlpool.tile([SPC, 1], f32, tag="e8p")
            nc.vector.scalar_tensor_tensor(e8p[:], dred8[:], -2.0, tbtf[:],
                                           OP.mult, OP.add)
            cA = smlpool.tile([128, 1], f32, tag="cA")
            nc.vector.scalar_tensor_tensor(cA[:], lnr2, wv[:], kconst[:],
                                           OP.mult, OP.add)
            e8b = smlpool.tile([SPC, 1], f32, tag="e8b")
            nc.vector.scalar_tensor_tensor(e8b[:], lnacc[0:SPC, 0:1],
                                           lnr2[0:SPC, 0:1], e8p[:],
                                           OP.subtract, OP.add)

            # both sums accumulate into ONE psum cell; the shift constant is
            # folded into the cA matmul via kconst (cA' = lnr2*wv + K/128)
            fin_ps = ps_a.tile([1, 1], f32, tag="a")
            nc.tensor.matmul(fin_ps[:], cA[:], onesf[:],
                             start=True, stop=False)
            nc.tensor.matmul(fin_ps[:], e8b[:], onesf[0:SPC, 0:1],
                             start=False, stop=True)
            res = smlpool.tile([1, 1], f32, tag="res")
            nc.vector.tensor_copy(res[:], fin_ps[:])
            nc.sync.dma_start(out_ext[:], res[:])
            if KDBG:
                nc.sync.dma_start(dbg_cmp[:], cmp[:])
                dbgw = smlpool.tile([128, 8], f32, tag="dbgw")
                nc.vector.tensor_copy(dbgw[:, 0:1], cntm[:])
                nc.vector.tensor_copy(dbgw[:, 1:2], wf[:])
                nc.vector.tensor_copy(dbgw[:, 2:3], wv[:])
                nc.vector.tensor_copy(dbgw[:, 3:4], notv[:])
                nc.vector.tensor_copy(dbgw[:, 4:5], sege[:])
                nc.vector.tensor_copy(dbgw[:, 5:6], keff[:])
                nc.vector.tensor_copy(dbgw[:, 6:7], rows_i32[:])
                nc.vector.tensor_copy(dbgw[:, 7:8], sv)
                nc.sync.dma_start(dbg_w[:], dbgw[:])
                nc.sync.dma_start(dbg_eq[:], eqm[:])
                nc.sync.dma_start(dbg_eg[:], eg[:])
                dbgc1 = smlpool.tile([SPC, P + 2], f32, tag="dbgc1")
                nc.vector.tensor_copy(dbgc1[:], c1all[0:SPC, :])
                nc.sync.dma_start(dbg_c1[:], dbgc1[:])
                nc.sync.dma_start(dbg_ex[:], exsum[:])
                dbgf = smlpool.tile([128, 8], f32, tag="dbgf")
                nc.vector.tensor_copy(dbgf[:, 0:1], r2comb[:])
                nc.vector.tensor_copy(dbgf[:, 1:2], lnpack[:, 0:1])
                nc.vector.tensor_copy(dbgf[:, 2:3], lnr2)
                nc.vector.tensor_copy(dbgf[:, 3:4], cA[:])
                nc.vector.tensor_copy(dbgf[0:SPC, 4:5], lnacc[0:SPC, 0:1])
                nc.vector.tensor_copy(dbgf[0:SPC, 5:6], dred8[:])
                nc.vector.tensor_copy(dbgf[0:SPC, 6:7], tbtf[:])
                nc.vector.tensor_copy(dbgf[0:1, 7:8], res[:])
                nc.sync.dma_start(dbg_fin[:], dbgf[:])

    nc.compile()
    _NC_CACHE = nc
    return nc


def _stage_core(te, ipe, bbox, am, c):
    """Build the in_map for core c from full inputs."""
    import ml_dtypes
    bf = ml_dtypes.bfloat16
    sl = slice(c * SPC, (c + 1) * SPC)
    ter = te[sl].reshape(SPC * L, C).astype(bf)
    ipet = (np.ascontiguousarray(ipe[sl]).transpose(0, 2, 1)   # [SPC, 768, 196]
            .reshape(SPC, 6, 128, P).transpose(2, 0, 1, 3)
            .reshape(128, SPC * 6 * P)).astype(bf)
    bbq = (bbox[sl].astype(np.int32).reshape(SPC, 128, TPD, 4)
           .transpose(1, 0, 2, 3).reshape(128, SPC * 16))
    mq = (am[sl].astype(np.float32).reshape(SPC, 128, TPD)
          .transpose(1, 0, 2).reshape(128, NTOK))
    return {
        "ter": np.ascontiguousarray(ter),
        "ipes": np.ascontiguousarray(ipet),
        "bboxq": np.ascontiguousarray(bbq),
        "maskq": np.ascontiguousarray(mq),
    }


def _check_capacity(bbox, am):
    d = (bbox.astype(np.int64) // 72)
    val = ((d[..., 0] == d[..., 2]) & (d[..., 1] == d[..., 3])
           & (am != 0))
    per_core = val.reshape(NCORES, SPC * L).sum(axis=1)
    assert per_core.max() <= NSLOT, (
        f"valid-token count {per_core.max()} exceeds slot capacity {NSLOT}")


def _install_profile_hook():
    """Wire the NTFF profile hook (the image's antenv lacks axon_hooks)."""
    import sys
    import types
    try:
        import antenv.axon_hooks  # noqa: F401
        return
    except ImportError:
        pass
    import antenv
    mod = types.ModuleType("antenv.axon_hooks")
    holder = {}
    mod.set_axon_ntff_profile_hook = lambda h: holder.__setitem__("h", h)
    mod.get_axon_ntff_profile_hook = lambda: holder.get("h")
    sys.modules["antenv.axon_hooks"] = mod
    antenv.axon_hooks = mod
    from trn_agent_boot.trn_boot import _ntff_profile_via_ctypes
    mod.set_axon_ntff_profile_hook(
        _ntff_profile_via_ctypes("/opt/axon/libaxon_pjrt.so"))
    import concourse.bass_utils as bu
    bu.upload_artifacts = lambda tmpdir: f"local:{tmpdir}"


def _run(inputs, trace=False, trace_kwargs=None):
    from concourse.bass_utils import run_bass_kernel_spmd
    if trace:
        _install_profile_hook()
    te = np.asarray(inputs["text_embeds"], dtype=np.float32)
    ipe = np.asarray(inputs["image_patch_embedding"], dtype=np.float32)
    bbox = np.asarray(inputs["bbox"])
    am = np.asarray(inputs["attention_mask"])
    _check_capacity(bbox, am)
    nc = _build_nc()
    in_maps = [_stage_core(te, ipe, bbox, am, c) for c in range(NCORES)]
    kw = {}
    if trace:
        kw = dict(trace=True, trace_kwargs=trace_kwargs or {})
    res = run_bass_kernel_spmd(nc, in_maps, core_ids=list(range(NCORES)), **kw)
    total = sum(float(res.results[i]["out"][0, 0]) for i in range(NCORES))
    loss = total / (2.0 * B * P)
    return np.asarray(loss, dtype=np.float32), res


def kernel(**inputs) -> np.ndarray:
    try:
        loss, _ = _run(inputs, trace=False)
    except Exception:
        # one retry: a previously wedged device recovers after a failed call
        loss, _ = _run(inputs, trace=False)
    return loss
